# revision 41
# baseline (speedup 1.0000x reference)
"""Multi-head causal attention (Whisper-style) on 8 trn2 NeuronCores.

Sharding: head-parallel (2 of 16 heads per core) + row-parallel output
projection.  Each core receives the full (host-transposed, bf16)
activations x^T, its column slice of Wq/Wk/Wv (+bias slices) and its row
slice of Wo, and produces a full-size partial y^T in bf16.  The host
sums the 8 partials in f64, transposes back, and adds bo.

bf16 is used only at the DMA boundary (x and weights in, y out) to halve
HBM traffic; GPSIMD upconverts to f32 on-chip so every matmul runs as
self-weight-loading f32r (a separate Ldweights per bf16 matmul would
serialize on the PE sequencer).

On-chip layout is fully "transposed" (feature dim on partitions):
  q^T,k^T = Wq/k^T-free matmuls over x^T -> [128, S] per batch
  scores^T[k,q] per (batch, k-chunk, q-tile) for BOTH heads at once
  (row-tiled matmuls at base partitions 0/64), one exp per slot covering
  both heads, softmax along partitions via a ones-column appended to v
  (denominator rides the o^T matmul, gets PE-broadcast to 64 partitions
  and divided out on DVE).
Mask handling is value-driven host-side: blocks are classified as
skip (fully masked), clean (no mask), diag (causal diagonal: a fully
masked column prefix of width o, then the canonical lower-triangle
128x128 window) or gen (anything else).  diag blocks compute only
columns [o:512) and multiply the 128-wide window by a single resident
0/1 triangle; gen blocks multiply by exp(mask) tiles.
"""

import os
import sys
from contextlib import ExitStack

import numpy as np

for _p in ("/root/.axon_site/_ro/trn_rl_repo", "/opt/trn_rl_repo"):
    if os.path.isdir(_p) and _p not in sys.path:
        sys.path.append(_p)

import concourse.bass as bass
import concourse.mybir as mybir
import concourse.tile as tile
from concourse import bacc, bass_utils

F32 = mybir.dt.float32
F32R = mybir.dt.float32r
BF16 = mybir.dt.bfloat16
NP_BF16 = mybir.dt.np(BF16)
AF = mybir.ActivationFunctionType
ALU = mybir.AluOpType

N_STATE = 1024
N_HEAD = 16
HD = 64
N_CORES = 8
HEADS_PER_CORE = N_HEAD // N_CORES  # 2
E = HEADS_PER_CORE * HD  # 128 feature columns per core
Q_TILE = 512
K_CHUNK = 128
N_D = N_STATE // 128  # 8 contraction chunks for the projections
SCALE2 = float(HD) ** -0.5  # 0.125, applied once on the q side
NEG_THRESH = -50.0


def classify_blocks(maskT):
    """Value-driven classification of (k_chunk, q_tile) mask blocks.

    Returns {(ki, j): ("skip"|"clean", None) | ("diag", o) | ("gen", None)}.
    "diag" means: columns [0,o) fully masked, columns [o,o+128) form the
    canonical lower-triangular window (element (k,c) valid iff k <= c-o),
    columns [o+128:) fully clean.
    """
    S = maskT.shape[0]
    kk = np.arange(K_CHUNK)[:, None]
    cls = {}
    for ki in range(S // K_CHUNK):
        for j in range(S // Q_TILE):
            blk = maskT[ki * K_CHUNK:(ki + 1) * K_CHUNK,
                        j * Q_TILE:(j + 1) * Q_TILE]
            neg = blk < NEG_THRESH
            zero = blk == 0.0
            if neg.all():
                cls[(ki, j)] = ("skip", None)
                continue
            if zero.all():
                cls[(ki, j)] = ("clean", None)
                continue
            fullcol = neg.all(axis=0)
            o = 0
            while o < Q_TILE and fullcol[o]:
                o += 1
            w = min(K_CHUNK, Q_TILE - o)
            tri = kk <= (np.arange(w)[None, :])
            diag_ok = (
                (zero[:, o:o + w] == tri).all()
                and (neg[:, o:o + w] == ~tri).all()
                and zero[:, o + w:].all()
            )
            cls[(ki, j)] = ("diag", o) if diag_ok else ("gen", None)

    # The first valid block of each q-tile seeds the o^T psum accumulation
    # and must cover the full tile width; demote to "gen" otherwise.
    for j in range(S // Q_TILE):
        for ki in range(S // K_CHUNK):
            kind, o = cls[(ki, j)]
            if kind == "skip":
                continue
            if kind == "diag" and o != 0:
                cls[(ki, j)] = ("gen", None)
            break
    return cls


def build_kernel(B, S, cls, repeats=1):
    """Build the per-core SPMD Bass program (identical on all cores)."""
    n_k = S // K_CHUNK
    n_q = S // Q_TILE
    valid = {j: [ki for ki in range(n_k) if cls[(ki, j)][0] != "skip"]
             for j in range(n_q)}
    gen_blocks = sorted(k for k, v in cls.items() if v[0] == "gen")
    gen_index = {blk: i for i, blk in enumerate(gen_blocks)}

    nc = bacc.Bacc("TRN2", target_bir_lowering=False, debug=False,
                   num_devices=N_CORES)

    xT_d = nc.dram_tensor("xT", [B, N_STATE, S], F32R, kind="ExternalInput")
    wq_d = nc.dram_tensor("wq", [128, N_D * E], BF16, kind="ExternalInput")
    wk_d = nc.dram_tensor("wk", [128, N_D * E], BF16, kind="ExternalInput")
    wv_d = nc.dram_tensor("wv", [128, N_D * E], BF16, kind="ExternalInput")
    wo_d = nc.dram_tensor("wo", [E, N_STATE], BF16, kind="ExternalInput")
    bq_d = nc.dram_tensor("bq", [E], F32, kind="ExternalInput")
    bv_d = nc.dram_tensor("bv", [E], F32, kind="ExternalInput")
    ident_d = nc.dram_tensor("ident", [128, 128], F32R, kind="ExternalInput")
    tri2_d = nc.dram_tensor("tri2", [K_CHUNK, 2 * K_CHUNK], F32,
                            kind="ExternalInput")
    ones2_d = nc.dram_tensor("ones2", [2, 128], F32R, kind="ExternalInput")
    if gen_blocks:
        gm_d = nc.dram_tensor("gm", [len(gen_blocks) * K_CHUNK, 2 * Q_TILE],
                              F32, kind="ExternalInput")
    yT_d = nc.dram_tensor("yT", [B, N_STATE, S], BF16, kind="ExternalOutput")

    with tile.TileContext(nc) as tc, ExitStack() as ctx:
        const = ctx.enter_context(tc.tile_pool(name="const", bufs=1))
        xpool = ctx.enter_context(tc.tile_pool(name="xpool", bufs=2))
        stage = ctx.enter_context(tc.tile_pool(name="stage", bufs=3))
        wexp = ctx.enter_context(tc.tile_pool(name="wexp", bufs=6))
        yspool = ctx.enter_context(tc.tile_pool(name="yspool", bufs=2))
        # PSUM: mm ring 2 x [128,2,512] f32 slots (2 banks each) plus one
        # dedicated [128,2,512] o^T accumulator slot per batch (head h in
        # bank h, rows 0:65) = all 8 banks.  The two batches' attention
        # blocks interleave, so each batch's accumulator lives across its
        # block while the other batch computes.
        psM = ctx.enter_context(tc.tile_pool(name="psM", bufs=2, space="PSUM"))
        psOt = ctx.enter_context(tc.tile_pool(name="psOt", bufs=1,
                                              space="PSUM"))

        # ---- resident constants / weights (bf16 staged, Pool-upconverted) --
        wbf = const.tile([128, 4, N_D * E], BF16, tag="wbf")
        for i, w_d in enumerate((wq_d, wk_d, wv_d)):
            nc.scalar.dma_start(wbf[:, i, :], w_d[:])
        nc.scalar.dma_start(wbf[:, 3, :], wo_d[:])
        wq_sb = const.tile([128, N_D, E], F32R, tag="wq_sb")
        wk_sb = const.tile([128, N_D, E], F32R, tag="wk_sb")
        wv_sb = const.tile([128, N_D, E], F32R, tag="wv_sb")
        wo_sb = const.tile([E, N_STATE], F32R, tag="wo_sb")
        for i, w_sb in enumerate((wq_sb, wk_sb, wv_sb)):
            nc.gpsimd.tensor_copy(
                w_sb[:].rearrange("p c e -> p (c e)"), wbf[:, i, :])
        nc.gpsimd.tensor_copy(wo_sb[:], wbf[:, 3, :])
        bq_sb = const.tile([E, 1], F32, tag="bq_sb")
        bv_sb = const.tile([E, 1], F32, tag="bv_sb")
        nc.scalar.dma_start(bq_sb[:], bq_d[:].rearrange("(e o) -> e o", o=1))
        nc.scalar.dma_start(bv_sb[:], bv_d[:].rearrange("(e o) -> e o", o=1))
        ident = const.tile([128, 128], F32R, tag="ident")
        nc.scalar.dma_start(ident[:], ident_d[:])
        tri2 = const.tile([K_CHUNK, 2, K_CHUNK], F32, tag="tri2")
        nc.scalar.dma_start(
            tri2[:], tri2_d[:].rearrange("p (a c) -> p a c", a=2))
        ones1 = const.tile([1, HD], F32R, tag="ones1")
        nc.scalar.dma_start(ones1[:], ones2_d[0:1, 0:HD])
        gm_sb = {}
        for blk in gen_blocks:
            gi = gen_index[blk]
            mt = const.tile([K_CHUNK, 2, Q_TILE], F32,
                            name=f"gm_{gi}", tag=f"gm_{gi}")
            nc.scalar.dma_start(
                mt[:], gm_d[gi * K_CHUNK:(gi + 1) * K_CHUNK, :]
                .rearrange("p (a q) -> p a q", a=2))
            gm_sb[blk] = mt

        # resident activations
        qT = [const.tile([E, S], F32R, name=f"qT{b}", tag=f"qT{b}")
              for b in range(B)]
        kT = [const.tile([E, S], F32R, name=f"kT{b}", tag=f"kT{b}")
              for b in range(B)]
        onT = [const.tile([E, S], F32R, name=f"onT{b}", tag=f"onT{b}")
               for b in range(B)]
        vn = [const.tile([128, n_k, 2 * (HD + 1)], F32R, name=f"vn{b}",
                         tag=f"vn{b}") for b in range(B)]
        for b in range(B):
            # ones columns (positions HD and 2HD+1) are written once; the
            # per-rep v copies never touch them.
            nc.vector.memset(vn[b][:].bitcast(F32), 1.0)

        # ---- stage A: projections for batch b, token tile j ----
        def a_units(b, inj=False):
            """Per-(j) sub-unit callables: [x, q, k, v] x n_q, in order."""
            units = []
            xts = {}

            def x_unit(j):
                ts = slice(j * Q_TILE, (j + 1) * Q_TILE)
                xt = xpool.tile([128, N_D, Q_TILE], F32R, tag="xt")
                src_ap = xT_d[b, :, ts].rearrange("(c p) t -> p c t", p=128)
                if j == 0:
                    # halve the first tile's transfer so the projections
                    # start sooner out of the pipeline head
                    h = N_D // 2
                    nc.sync.dma_start(xt[:, 0:h, :], src_ap[:, 0:h, :])
                    nc.sync.dma_start(xt[:, h:, :], src_ap[:, h:, :])
                else:
                    nc.sync.dma_start(xt[:], src_ap)
                xts[j] = xt

            def proj_unit(j, proj):
                ts = slice(j * Q_TILE, (j + 1) * Q_TILE)
                xt = xts[j]
                w_sb = {"q": wq_sb, "k": wk_sb, "v": wv_sb}[proj]
                # injected units borrow this batch's idle o^T slot so the
                # mm ring keeps double-buffering the enclosing block's scores
                if inj:
                    ps2 = psOt.tile([128, 2, Q_TILE], F32, tag=f"ot{b}",
                                    name=f"ps{b}_{j}_{proj}")
                else:
                    ps2 = psM.tile([128, 2, Q_TILE], F32, tag="mm")
                ps = ps2[:, 0, :]
                for c in range(N_D):
                    nc.tensor.matmul(ps, w_sb[:, c, :], xt[:, c, :],
                                     start=(c == 0), stop=(c == N_D - 1))
                if proj == "q":
                    nc.vector.tensor_scalar(
                        qT[b][:, ts], ps, bq_sb[:], SCALE2, ALU.add, ALU.mult)
                elif proj == "k":
                    if b == 0 and j == 0:
                        nc.scalar.activation(kT[b][:, ts], ps, AF.Copy)
                    else:
                        nc.vector.tensor_copy(kT[b][:, ts], ps)
                else:
                    vs = stage.tile([E, Q_TILE], F32R, tag="vs")
                    nc.vector.tensor_scalar(vs[:], ps, bv_sb[:], None, ALU.add)
                    for c in range(Q_TILE // 128):
                        if inj:
                            tp2 = psOt.tile([128, 2, Q_TILE], F32R,
                                            tag=f"ot{b}",
                                            name=f"tp{b}_{j}_{c}")
                        else:
                            tp2 = psM.tile([128, 2, Q_TILE], F32R, tag="mm")
                        tp = tp2[:, 0, 0:128]
                        nc.tensor.matmul(tp, vs[:, c * 128:(c + 1) * 128],
                                         ident[:], is_transpose=True)
                        ci = j * (Q_TILE // 128) + c
                        # strided copy drops the transposed [tok, feat] tile
                        # around the resident ones columns in one shot
                        dst = vn[b][:, ci, :]
                        dstap = bass.AP(dst.tensor, dst.offset,
                                        [list(dst.ap[0]), [HD + 1, 2],
                                         [1, HD]])
                        nc.vector.tensor_copy(dstap, tp[:, 0:128])

            for j in range(n_q):
                units.append((lambda jj: (lambda: x_unit(jj)))(j))
                for proj in ("q", "k", "v"):
                    units.append(
                        (lambda jj, pp: (lambda: proj_unit(jj, pp)))(j, proj))
            return units

        # ---- stage B: attention for batch b, q-tile j (both heads) ----
        def b_block(b, j, inject=None, fin_prev=None, last=False):
            inject = list(inject or ())
            vkis = valid[j]
            fin_at = min(2, len(vkis) - 1)
            qs0 = j * Q_TILE
            otp = psOt.tile([128, 2, Q_TILE], F32, tag=f"ot{b}",
                            name=f"ot{b}_{j}")
            ot = [otp[0:HD + 1, h, :] for h in range(2)]

            def emit_ot(ki, o, wt2):
                last = ki == vkis[-1]
                first = ki == vkis[0]
                for h in range(2):
                    vsl = slice(h * (HD + 1), (h + 1) * (HD + 1))
                    nc.tensor.matmul(ot[h][:, o:], vn[b][:, ki, vsl],
                                     wt2[:, h, o:], start=first, stop=last)

            # injected units may read tensors written by fin_prev, so
            # pops may only start after the fin has been emitted
            start_at = (fin_at + 1) if fin_prev is not None else 0
            n_pop = max(1, len(vkis) - start_at)
            quota = max(1, -(-len(inject) // n_pop)) if inject else 0
            pending = []  # 2-deep SW pipeline: o^T lags scores by 2 slots
            for idx, ki in enumerate(vkis):
                if fin_prev is not None and idx == fin_at:
                    fin_prev()
                    fin_prev = None
                if idx >= start_at:
                    for _ in range(quota):
                        if inject:
                            inject.pop(0)()
                kind, o = cls[(ki, j)]
                o = o or 0
                ks = slice(ki * K_CHUNK, (ki + 1) * K_CHUNK)
                sc2 = psM.tile([128, 2, Q_TILE], F32, tag="mm")
                for h in range(2):
                    hs = slice(h * HD, (h + 1) * HD)
                    nc.tensor.matmul(sc2[:, h, o:], kT[b][hs, ks],
                                     qT[b][hs, qs0 + o:qs0 + Q_TILE],
                                     start=True, stop=True)
                if len(pending) >= 2:
                    emit_ot(*pending.pop(0))
                wt2 = wexp.tile([K_CHUNK, 2, Q_TILE], F32R, tag="wexp")
                nc.scalar.activation(wt2[:, :, o:], sc2[:, :, o:], AF.Exp)
                if kind == "diag":
                    nc.gpsimd.tensor_tensor(
                        wt2[:, :, o:o + K_CHUNK], wt2[:, :, o:o + K_CHUNK],
                        tri2[:], ALU.mult)
                elif kind == "gen":
                    nc.gpsimd.tensor_tensor(
                        wt2[:, :, :], wt2[:, :, :], gm_sb[(ki, j)][:],
                        ALU.mult)
                pending.append((ki, o, wt2))
            while pending:
                emit_ot(*pending.pop(0))
            if fin_prev is not None:
                fin_prev()
            while inject:
                inject.pop(0)()

            def finish():
                # normalize: per-head PE-broadcast of the denominator row,
                # then onT = o / d (ALU divide)
                bc2p = psM.tile([128, 2, Q_TILE], F32, tag="mm",
                                name=f"bc{b}_{j}")
                for h in range(2):
                    dh = stage.tile([1, Q_TILE], F32R, tag="dh",
                                    name=f"dh_{b}_{j}_{h}")
                    with nc.allow_low_precision(
                            reason="f32r is bit-identical to f32"):
                        nc.vector.reciprocal(dh[:], ot[h][HD:HD + 1, :])
                    nc.tensor.matmul(bc2p[0:HD, h, :], ones1[:], dh[:],
                                     start=True, stop=True)
                bcs = stage.tile([HD, 2, Q_TILE], F32, tag="bcs",
                                 name=f"bcs_{b}_{j}")
                if last:
                    nc.scalar.activation(bcs[:], bc2p[0:HD, :, :], AF.Copy)
                else:
                    nc.vector.tensor_copy(bcs[:], bc2p[0:HD, :, :])
                for h in range(2):
                    hs = slice(h * HD, (h + 1) * HD)
                    nc.vector.tensor_tensor(
                        onT[b][hs, qs0:qs0 + Q_TILE], ot[h][0:HD, :],
                        bcs[:, h, :], ALU.mult)
            return finish

        # ---- stage C: output projection; one big y store per (b, jp) ----
        def c_units(b, jp, store_eng=None, tail=False):
            units = []
            ysb = {}

            def c_unit(m):
                if not ysb:
                    ysb[0] = yspool.tile([128, N_D, 2 * Q_TILE], BF16,
                                         name=f"ysbig{b}_{jp}", tag="ysbig")
                ms = slice(m * 128, (m + 1) * 128)
                # ride batch b's own o^T psum slot: it is idle while the
                # other batch's attention block (which we are injected
                # into) is running.  Tail groups also alternate with the
                # mm ring (free by then) to halve slot serialization.
                yp2 = psOt.tile([128, 2, Q_TILE], F32, tag=f"ot{b}",
                                name=f"yp{b}_{jp}_{m}")
                for jj in range(2):
                    qs = slice((2 * jp + jj) * Q_TILE,
                               (2 * jp + jj + 1) * Q_TILE)
                    nc.tensor.matmul(yp2[:, jj, :], wo_sb[:, ms],
                                     onT[b][:, qs], start=True, stop=True)
                if tail and m % 2 == 1:
                    nc.scalar.activation(
                        ysb[0][:, m, :], yp2[:].rearrange("p a q -> p (a q)"),
                        AF.Copy)
                else:
                    nc.vector.tensor_copy(
                        ysb[0][:, m, :], yp2[:].rearrange("p a q -> p (a q)"))

            def store_half(half):
                eng = store_eng or nc.sync
                hm = slice(half * (N_D // 2), (half + 1) * (N_D // 2))
                eng.dma_start(
                    yT_d[b, half * (N_STATE // 2):(half + 1) * (N_STATE // 2),
                         2 * jp * Q_TILE:(2 * jp + 2) * Q_TILE]
                    .rearrange("(c p) t -> p c t", p=128), ysb[0][:, hm, :])

            for m in range(N_STATE // 128):
                units.append((lambda mm_: (lambda: c_unit(mm_)))(m))
                if tail and m == N_D // 2 - 1:
                    units.append(lambda: store_half(0))
            if tail:
                units.append(lambda: store_half(1))
            else:
                units.append(lambda: (store_half(0), store_half(1)))
            return units

        # ---- emission schedule ----
        a0 = a_units(0)
        first_rep = True
        for _rep in range(repeats):
            a1 = a_units(1, inj=True)
            # The two batches' attention blocks interleave so every block
            # boundary has the other batch's independent work in flight;
            # each block's normalize chain is deferred into the next
            # emitted block's slot stream.
            def emit_a(units):
                for u in units:
                    u()

            if first_rep:
                a0[0]()  # later reps prefetch this at the prior rep's tail
                first_rep = False
            emit_a(a0[1:4])
            emit_a(a1[0:4])
            f00 = b_block(0, 0)
            emit_a(a0[4:8])
            emit_a(a1[4:8])
            f10 = b_block(1, 0, fin_prev=f00)
            emit_a(a0[8:12])
            emit_a(a1[8:12])
            f01 = b_block(0, 1, fin_prev=f10)
            emit_a(a0[12:16])
            emit_a(a1[12:16])
            c0a = c_units(0, 0)
            c1a = c_units(1, 0)
            c0b = c_units(0, 1)
            # batch-b C units may only be injected into batch-(1-b) blocks
            # (they borrow batch b's o^T psum slot)
            f11 = b_block(1, 1, fin_prev=f01, inject=c0a[:5])
            f02 = b_block(0, 2, fin_prev=f11, inject=c1a[:5])
            f12 = b_block(1, 2, fin_prev=f02, inject=c0a[5:])
            f03 = b_block(0, 3, fin_prev=f12, inject=c1a[5:])
            a0_next = a_units(0) if _rep + 1 < repeats else None
            if a0_next is not None:
                # next rep's first x tile rides ahead of the last block's
                # stores and the tail
                a0_next[0]()
            f13 = b_block(1, 3, fin_prev=f03, inject=c0b, last=True)
            f13()
            for u in c_units(1, 1, tail=True):
                u()
            a0 = a0_next

    nc.finalize()
    return nc


def _prep_w(w):
    """[1024, 128] slice -> [128, N_D*128] bf16 with [p, c, e] layout."""
    return np.ascontiguousarray(
        w.reshape(N_D, 128, E).transpose(1, 0, 2).reshape(128, N_D * E)
    ).astype(NP_BF16)


def shard_inputs(x, mask, Wq, bq, Wk, Wv, bv, Wo, cls):
    """Per-core input dicts (host-side layout prep + slicing only)."""
    xT = np.ascontiguousarray(
        x.transpose(0, 2, 1)).astype(np.float32)
    tri = (np.arange(K_CHUNK)[:, None] <= np.arange(K_CHUNK)[None, :])
    tri2 = np.ascontiguousarray(
        np.concatenate([tri, tri], axis=1)).astype(np.float32)
    ones2 = np.zeros((2, 128), dtype=np.float32)
    ones2[0, :HD] = 1.0
    ones2[1, HD:] = 1.0
    ident = np.eye(128, dtype=np.float32)

    gen_blocks = sorted(k for k, v in cls.items() if v[0] == "gen")
    gm = None
    if gen_blocks:
        maskT = np.ascontiguousarray(mask.T).astype(np.float64)
        tiles = []
        for (ki, j) in gen_blocks:
            blk = maskT[ki * K_CHUNK:(ki + 1) * K_CHUNK,
                        j * Q_TILE:(j + 1) * Q_TILE]
            em = np.exp(blk).astype(np.float32)
            tiles.append(np.concatenate([em, em], axis=1))
        gm = np.ascontiguousarray(np.concatenate(tiles, axis=0))

    in_maps = []
    for c in range(N_CORES):
        cs = slice(c * E, (c + 1) * E)
        m = {
            "xT": xT,
            "wq": _prep_w(np.ascontiguousarray(Wq[:, cs])),
            "wk": _prep_w(np.ascontiguousarray(Wk[:, cs])),
            "wv": _prep_w(np.ascontiguousarray(Wv[:, cs])),
            "wo": np.ascontiguousarray(Wo[cs, :]).astype(NP_BF16),
            "bq": np.ascontiguousarray(bq[cs]).astype(np.float32),
            "bv": np.ascontiguousarray(bv[cs]).astype(np.float32),
            "ident": ident,
            "tri2": tri2,
            "ones2": ones2,
        }
        if gm is not None:
            m["gm"] = gm
        in_maps.append(m)
    return in_maps


_NC_CACHE = {}


def _get_nc(B, S, cls_key, cls, repeats=1):
    key = (B, S, cls_key, repeats)
    if key not in _NC_CACHE:
        _NC_CACHE[key] = build_kernel(B, S, cls, repeats=repeats)
    return _NC_CACHE[key]


def _classify(mask):
    maskT = np.ascontiguousarray(np.asarray(mask).T).astype(np.float32)
    cls = classify_blocks(maskT)
    cls_key = tuple(sorted(cls.items()))
    return cls, cls_key


def run(x, mask, Wq, bq, Wk, Wv, bv, Wo, bo, trace=False):
    B, S, D = x.shape
    cls, cls_key = _classify(mask)
    nc = _get_nc(B, S, hash(cls_key), cls)
    in_maps = shard_inputs(np.asarray(x, np.float32), np.asarray(mask),
                           np.asarray(Wq, np.float32), np.asarray(bq, np.float32),
                           np.asarray(Wk, np.float32), np.asarray(Wv, np.float32),
                           np.asarray(bv, np.float32), np.asarray(Wo, np.float32),
                           cls)
    res = bass_utils.run_bass_kernel_spmd(
        nc, in_maps, core_ids=list(range(N_CORES)), trace=trace)
    acc = np.zeros((B, N_STATE, S), dtype=np.float64)
    for r in res.results:
        acc += r["yT"].astype(np.float64)
    y = (acc.transpose(0, 2, 1) + np.asarray(bo, np.float64)).astype(np.float32)
    return y, res


def kernel(x, mask, Wq, bq, Wk, Wv, bv, Wo, bo):
    y, _ = run(x, mask, Wq, bq, Wk, Wv, bv, Wo, bo, trace=False)
    return y


def time_run(x, mask, Wq, bq, Wk, Wv, bv, Wo, bo, iters=20, repeats=1):
    """Measure per-iteration device execution time of the SPMD program.

    Mirrors bass2jax.run_bass_via_pjrt's multi-core lowering, but keeps
    inputs device-resident and chains donated output buffers so `iters`
    executions pipeline back-to-back; returns (y, seconds_per_iter).
    """
    import time as _time
    import jax
    from jax.experimental.shard_map import shard_map
    from jax.sharding import Mesh, NamedSharding, PartitionSpec
    from concourse import bass2jax
    from concourse.bass2jax import _bass_exec_p, install_neuronx_cc_hook

    install_neuronx_cc_hook()
    B, S, D = x.shape
    cls, cls_key = _classify(mask)
    nc = _get_nc(B, S, hash(cls_key), cls, repeats=repeats)
    in_maps = shard_inputs(np.asarray(x, np.float32), np.asarray(mask),
                           np.asarray(Wq, np.float32), np.asarray(bq, np.float32),
                           np.asarray(Wk, np.float32), np.asarray(Wv, np.float32),
                           np.asarray(bv, np.float32), np.asarray(Wo, np.float32),
                           cls)

    in_names, out_names, out_avals, zero_outs = [], [], [], []
    partition_name = (nc.partition_id_tensor.name
                      if nc.partition_id_tensor else None)
    for alloc in nc.m.functions[0].allocations:
        if not isinstance(alloc, mybir.MemoryLocationSet):
            continue
        name = alloc.memorylocations[0].name
        if alloc.kind == "ExternalInput":
            if name != partition_name:
                in_names.append(name)
        elif alloc.kind == "ExternalOutput":
            out_names.append(name)
            shape = tuple(alloc.tensor_shape)
            dtype = mybir.dt.np(alloc.dtype)
            out_avals.append((shape, dtype))
            zero_outs.append(np.zeros(shape, dtype))
    n_params = len(in_names)
    n_outs = len(out_names)
    all_in_names = list(in_names) + list(out_names)
    if partition_name is not None:
        all_in_names.append(partition_name)

    def _body(*args):
        operands = list(args)
        if partition_name is not None:
            operands.append(bass2jax.partition_id_tensor())
        outs = _bass_exec_p.bind(
            *operands,
            out_avals=tuple(
                jax.core.ShapedArray(s, d) for s, d in out_avals),
            in_names=tuple(all_in_names),
            out_names=tuple(out_names),
            lowering_input_output_aliases=(),
            sim_require_finite=True,
            sim_require_nnan=True,
            nc=nc,
        )
        return tuple(outs)

    devices = jax.devices()[:N_CORES]
    mesh = Mesh(np.asarray(devices), ("core",))
    spec = PartitionSpec("core")
    donate = tuple(range(n_params, n_params + n_outs))
    sharded = jax.jit(
        shard_map(_body, mesh=mesh, in_specs=(spec,) * (n_params + n_outs),
                  out_specs=(spec,) * n_outs, check_rep=False),
        donate_argnums=donate, keep_unused=True)

    sh = NamedSharding(mesh, spec)
    dev_in = [
        jax.device_put(
            np.concatenate([np.asarray(in_maps[c][nm]) for c in range(N_CORES)],
                           axis=0), sh)
        for nm in in_names
    ]
    out = sharded(*dev_in, *[
        jax.device_put(np.zeros((N_CORES * z.shape[0], *z.shape[1:]), z.dtype),
                       sh) for z in zero_outs])
    jax.block_until_ready(out)  # warmup + compile
    t0 = _time.perf_counter()
    for _ in range(iters):
        out = sharded(*dev_in, *out)
    jax.block_until_ready(out)
    dt = (_time.perf_counter() - t0) / iters

    yT_all = np.asarray(out[out_names.index("yT")])
    acc = np.zeros((B, N_STATE, S), dtype=np.float64)
    for c in range(N_CORES):
        acc += yT_all.reshape(N_CORES, B, N_STATE, S)[c].astype(np.float64)
    y = (acc.transpose(0, 2, 1) + np.asarray(bo, np.float64)).astype(np.float32)
    return y, dt
